# revision 23
# baseline (speedup 1.0000x reference)
"""Trainium2 Bass kernel for causal attention (scores = K @ Q^T variant).

Problem (hardcoded):
  x  [8, 2048, 2048] f32, Wk/Wq/Wv [2048, 256] f32
  per batch b: K = x_b @ Wk, Q = x_b @ Wq, V = x_b @ Wv
  w = K @ Q^T / sqrt(256), causal-masked (strict upper = -inf),
  attn = softmax(w, axis=-1), out_b = attn @ V    -> [8, 2048, 256] f32

Sharding: data-parallel over batch, one batch element per NeuronCore (8 cores).
Compute dtype: float32r (single-pass PE matmul on fp32 bits, ~13 mantissa bits).
"""
import sys

for _p in ("/opt/trn_rl_repo",):
    if _p not in sys.path:
        sys.path.insert(0, _p)

import numpy as np

import concourse.bass as bass  # noqa: F401  (registers AP machinery)
import concourse.mybir as mybir
from concourse import bacc
from concourse.tile import TileContext
from concourse.bass_utils import run_bass_kernel_spmd
from concourse.masks import make_identity

F32 = mybir.dt.float32
F32R = mybir.dt.float32r

P = 128          # partitions
T = 2048         # sequence length (== E by construction of the module)
E = 2048         # embedding dim
D = 256          # head dim
EC = E // P      # 16 e-chunks
NT = T // P      # 16 t tiles
TB = 512         # stage-1 t-block width
NTB = T // TB    # 4
SB = 512         # stage-2 s-block width
SCALE = 1.0 / 16.0   # 1/sqrt(D)
MASKVAL = -1e9

N_CORES = 8


def _build():
    nc = bacc.Bacc("TRN2", target_bir_lowering=False, debug=False,
                   num_devices=N_CORES)
    x_h = nc.dram_tensor("x", [T, E], F32, kind="ExternalInput")
    wk_h = nc.dram_tensor("Wk", [E, D], F32, kind="ExternalInput")
    wq_h = nc.dram_tensor("Wq", [E, D], F32, kind="ExternalInput")
    wv_h = nc.dram_tensor("Wv", [E, D], F32, kind="ExternalInput")
    y_h = nc.dram_tensor("out", [T, D], F32, kind="ExternalOutput")
    x_ap, y_ap = x_h.ap(), y_h.ap()

    with TileContext(nc) as tc:
        with tc.tile_pool(name="persist", bufs=1) as persist:
            # --- persistent tensors -------------------------------------
            # weights, f32r-rounded during the (SWDGE cast) DMA itself.
            # Interleave half-tensor chunks in consumption order (K, Q, V) so
            # Wk lands first on the serial SWDGE stream.
            w_r = []
            for name in ("wk_r", "wq_r", "wv_r"):
                w_r.append(persist.tile([P, EC, D], F32R, name=name))
            wk_r, wq_r, wv_r = w_r
            # SWDGE stream in consumption order (K, Q, V)
            for wt, h in ((wk_r, wk_h), (wq_r, wq_h), (wv_r, wv_h)):
                nc.gpsimd.dma_start(wt[:],
                                    h.ap().rearrange("(ec p) d -> p ec d", p=P))

            kt = persist.tile([P, 2, T], F32R, name="kt")     # K^T [d, t]
            qt = persist.tile([P, 2, T], F32R, name="qt")     # Q^T [d, s]
            v_sb = persist.tile([P, NT, D], F32R, name="v_sb")  # V [s, d]

            ident_f = persist.tile([P, P], F32, name="ident_f")
            make_identity(nc, ident_f[:])
            ident_r = persist.tile([P, P], F32R, name="ident_r")
            nc.vector.tensor_copy(ident_r[:], ident_f[:])

            # master causal mask [P, 1024]: cols 0:384 valid, 384:512 causal
            # triangle (row p valid for col j-384 <= p), 512:1024 masked.
            mask = persist.tile([P, 1024], F32, name="mask")
            nc.vector.memset(mask[:, 0:512], 0.0)
            nc.gpsimd.affine_select(
                out=mask[:, 384:512], in_=mask[:, 384:512],
                compare_op=mybir.AluOpType.is_ge, fill=MASKVAL,
                base=0, pattern=[[-1, P]], channel_multiplier=1,
            )
            nc.vector.memset(mask[:, 512:1024], MASKVAL)

            copy_idx = 0

            def copy(out_ap, in_ap):
                # alternate PSUM->SBUF copies between DVE and ACT
                nonlocal copy_idx
                copy_idx += 1
                if copy_idx % 2:
                    nc.vector.tensor_copy(out_ap, in_ap)
                else:
                    nc.scalar.copy(out_ap, in_ap)

            # --- stage 1: transpose x, project K^T/Q^T/V ----------------
            # Software-pipelined: tb+1's x-transposes are emitted interleaved
            # between tb's projection psum-groups so the PE never waits at a
            # t-block boundary. Stage-1 PSUM->SBUF copies run on DVE only
            # (ACT's instruction stream carries the x DMA issues).
            with tc.tile_pool(name="s1", bufs=1) as s1, \
                 tc.tile_pool(name="s1ps", bufs=1, space="PSUM") as s1ps:

                xtbs = {}

                def tr_units(tb):
                    """Yield 16 transpose units (4 MMs + 1 copy each)."""
                    xtb = xtbs[tb]
                    for ti in range(TB // P):
                        tt = tb * (TB // P) + ti
                        on_swdge = tt >= 12
                        x_dt = F32R if on_swdge else F32
                        x_id = ident_r if on_swdge else ident_f
                        x_t = s1.tile([P, E], x_dt, name="x_t", tag="x_t",
                                      bufs=3)
                        if on_swdge:
                            nc.gpsimd.dma_start(x_t[:],
                                                x_ap[tt * P:(tt + 1) * P, :])
                        elif tt == 3:
                            # one early tile on the otherwise-idle SP queue:
                            # only its first DMA issues promptly at boot
                            nc.sync.dma_start(x_t[:],
                                              x_ap[tt * P:(tt + 1) * P, :])
                        else:
                            # quarter-split so transposes of each 512-col
                            # group start as soon as that slice lands
                            for q4 in range(4):
                                nc.scalar.dma_start(
                                    x_t[:, q4 * 512:(q4 + 1) * 512],
                                    x_ap[tt * P:(tt + 1) * P,
                                         q4 * 512:(q4 + 1) * 512])
                        for ecg in range(EC // 4):
                            tr_ps = s1ps.tile([P, 4, P], x_dt, name="tr_ps",
                                              tag="tr", bufs=4)
                            for j in range(4):
                                ec = ecg * 4 + j
                                nc.tensor.transpose(
                                    tr_ps[:, j],
                                    x_t[:, ec * P:(ec + 1) * P],
                                    x_id[:])
                            nc.vector.tensor_copy(
                                xtb[:, ecg * 4:(ecg + 1) * 4,
                                    ti * P:(ti + 1) * P],
                                tr_ps[:])
                            yield

                def proj_units(tb):
                    """Yield 8 projection units (16 MMs + 1 copy each)."""
                    xtb = xtbs[tb]
                    for wt, dst in ((wk_r, kt), (wq_r, qt)):
                        for dc in range(2):
                            pp = s1ps.tile([P, TB], F32, name="pp",
                                           tag="proj", bufs=4)
                            for ec in range(EC):
                                nc.tensor.matmul(
                                    pp[:],
                                    wt[:, ec, dc * P:(dc + 1) * P],
                                    xtb[:, ec, :],
                                    start=(ec == 0), stop=(ec == EC - 1))
                            nc.vector.tensor_copy(
                                dst[:, dc, tb * TB:(tb + 1) * TB], pp[:])
                            yield
                    for st in range(TB // P):
                        sg = tb * (TB // P) + st
                        pv = s1ps.tile([P, D], F32, name="pv",
                                       tag="proj", bufs=4)
                        for ec in range(EC):
                            nc.tensor.matmul(
                                pv[:],
                                xtb[:, ec, st * P:(st + 1) * P],
                                wv_r[:, ec, :],
                                start=(ec == 0), stop=(ec == EC - 1))
                        nc.vector.tensor_copy(v_sb[:, sg, :], pv[:])
                        yield

                xtbs[0] = s1.tile([P, EC, TB], F32R, name="xtb", tag="xtb",
                                  bufs=2)
                for _ in tr_units(0):
                    pass
                for tb in range(NTB):
                    if tb + 1 < NTB:
                        xtbs[tb + 1] = s1.tile([P, EC, TB], F32R, name="xtb",
                                               tag="xtb", bufs=2)
                        nxt = tr_units(tb + 1)
                    else:
                        nxt = iter(())
                    for _ in proj_units(tb):
                        next(nxt, None)
                        next(nxt, None)
                    for _ in nxt:
                        pass

            # --- stage 2: causal attention per 128-row query tile -------
            with tc.tile_pool(name="s2", bufs=1) as s2, \
                 tc.tile_pool(name="s2ps", bufs=1, space="PSUM") as s2ps:
                for tt in range(NT):
                    nblk = tt // 4 + 1
                    rem = (tt + 1) * P - (nblk - 1) * SB   # 128..512
                    out_ps = s2ps.tile([P, D], F32, name="out_ps", tag="out",
                                       bufs=2)
                    sums = s2.tile([P, 4], F32, name="sums", tag="sums",
                                   bufs=2)
                    for b in range(nblk):
                        s0 = b * SB
                        last = b == nblk - 1
                        nj = (rem // P) if last else (SB // P)
                        sc_ps = s2ps.tile([P, SB], F32, name="sc_ps",
                                          tag="sc", bufs=3)
                        for dc in range(2):
                            nc.tensor.matmul(
                                sc_ps[:],
                                kt[:, dc, tt * P:(tt + 1) * P],
                                qt[:, dc, s0:s0 + SB],
                                start=(dc == 0), stop=(dc == 1))
                        if last:
                            off = SB - rem
                            nc.vector.tensor_add(sc_ps[:], sc_ps[:],
                                                 mask[:, off:off + SB])
                        p_sb = s2.tile([P, SB], F32R, name="p_sb", tag="p",
                                       bufs=4)
                        nc.scalar.activation(
                            p_sb[:], sc_ps[:],
                            mybir.ActivationFunctionType.Exp,
                            scale=SCALE, accum_out=sums[:, b:b + 1])
                        pt_ps = s2ps.tile([P, 4, P], F32R, name="pt_ps",
                                          tag="pt", bufs=3)
                        for j in range(nj):
                            nc.tensor.transpose(
                                pt_ps[:, j], p_sb[:, j * P:(j + 1) * P],
                                ident_r[:])
                        pt_sb = s2.tile([P, 4, P], F32R, name="pt_sb",
                                        tag="pts", bufs=4)
                        copy(pt_sb[:, 0:nj], pt_ps[:, 0:nj])
                        for j in range(nj):
                            nc.tensor.matmul(
                                out_ps[:], pt_sb[:, j],
                                v_sb[:, b * 4 + j, :],
                                start=(b == 0 and j == 0),
                                stop=(last and j == nj - 1))
                    tot = s2.tile([P, 1], F32, name="tot", tag="tot", bufs=2)
                    nc.vector.reduce_sum(tot[:], sums[:, 0:nblk],
                                         axis=mybir.AxisListType.X)
                    rec = s2.tile([P, 1], F32, name="rec", tag="rec", bufs=2)
                    nc.vector.reciprocal(rec[:], tot[:])
                    o_sb = s2.tile([P, D], F32, name="o_sb", tag="osb", bufs=3)
                    nc.vector.tensor_scalar_mul(o_sb[:], out_ps[:], rec[:])
                    nc.sync.dma_start(y_ap[tt * P:(tt + 1) * P, :], o_sb[:])

    nc.compile()
    return nc


_NC_CACHE = None


def _get_nc():
    global _NC_CACHE
    if _NC_CACHE is None:
        _NC_CACHE = _build()
    return _NC_CACHE


def run(inputs: dict, trace: bool = False):
    """Run on 8 NeuronCores. Returns (out [8,T,D] f32, exec_time_ns|None)."""
    x = np.ascontiguousarray(np.asarray(inputs["x"], dtype=np.float32))
    wk = np.ascontiguousarray(np.asarray(inputs["Wk"], dtype=np.float32))
    wq = np.ascontiguousarray(np.asarray(inputs["Wq"], dtype=np.float32))
    wv = np.ascontiguousarray(np.asarray(inputs["Wv"], dtype=np.float32))
    assert x.shape == (N_CORES, T, E), x.shape

    nc = _get_nc()
    in_maps = [{"x": x[i], "Wk": wk, "Wq": wq, "Wv": wv}
               for i in range(N_CORES)]
    res = run_bass_kernel_spmd(nc, in_maps, core_ids=list(range(N_CORES)),
                               trace=trace)
    out = np.stack([res.results[i]["out"] for i in range(N_CORES)], axis=0)
    return out, res.exec_time_ns


def kernel(**inputs) -> np.ndarray:
    out, _ = run(inputs, trace=False)
    return out


# revision 24
# speedup vs baseline: 1.0066x; 1.0066x over previous
"""Trainium2 Bass kernel for causal attention (scores = K @ Q^T variant).

Problem (hardcoded):
  x  [8, 2048, 2048] f32, Wk/Wq/Wv [2048, 256] f32
  per batch b: K = x_b @ Wk, Q = x_b @ Wq, V = x_b @ Wv
  w = K @ Q^T / sqrt(256), causal-masked (strict upper = -inf),
  attn = softmax(w, axis=-1), out_b = attn @ V    -> [8, 2048, 256] f32

Sharding: data-parallel over batch, one batch element per NeuronCore (8 cores).
Compute dtype: float32r (single-pass PE matmul on fp32 bits, ~13 mantissa bits).
"""
import sys

for _p in ("/opt/trn_rl_repo",):
    if _p not in sys.path:
        sys.path.insert(0, _p)

import numpy as np

import concourse.bass as bass  # noqa: F401  (registers AP machinery)
import concourse.mybir as mybir
from concourse import bacc
from concourse.tile import TileContext
from concourse.bass_utils import run_bass_kernel_spmd
from concourse.masks import make_identity

F32 = mybir.dt.float32
F32R = mybir.dt.float32r

P = 128          # partitions
T = 2048         # sequence length (== E by construction of the module)
E = 2048         # embedding dim
D = 256          # head dim
EC = E // P      # 16 e-chunks
NT = T // P      # 16 t tiles
TB = 512         # stage-1 t-block width
NTB = T // TB    # 4
SB = 512         # stage-2 s-block width
SCALE = 1.0 / 16.0   # 1/sqrt(D)
MASKVAL = -1e9

N_CORES = 8


def _build():
    nc = bacc.Bacc("TRN2", target_bir_lowering=False, debug=False,
                   num_devices=N_CORES)
    x_h = nc.dram_tensor("x", [T, E], F32, kind="ExternalInput")
    wk_h = nc.dram_tensor("Wk", [E, D], F32, kind="ExternalInput")
    wq_h = nc.dram_tensor("Wq", [E, D], F32, kind="ExternalInput")
    wv_h = nc.dram_tensor("Wv", [E, D], F32, kind="ExternalInput")
    y_h = nc.dram_tensor("out", [T, D], F32, kind="ExternalOutput")
    x_ap, y_ap = x_h.ap(), y_h.ap()

    with TileContext(nc) as tc:
        with tc.tile_pool(name="persist", bufs=1) as persist:
            # --- persistent tensors -------------------------------------
            # weights, f32r-rounded during the (SWDGE cast) DMA itself.
            # Interleave half-tensor chunks in consumption order (K, Q, V) so
            # Wk lands first on the serial SWDGE stream.
            w_r = []
            for name in ("wk_r", "wq_r", "wv_r"):
                w_r.append(persist.tile([P, EC, D], F32R, name=name))
            wk_r, wq_r, wv_r = w_r
            # SWDGE stream in consumption order (K, Q, V)
            for wt, h in ((wk_r, wk_h), (wq_r, wq_h), (wv_r, wv_h)):
                nc.gpsimd.dma_start(wt[:],
                                    h.ap().rearrange("(ec p) d -> p ec d", p=P))

            kt = persist.tile([P, 2, T], F32R, name="kt")     # K^T [d, t]
            qt = persist.tile([P, 2, T], F32R, name="qt")     # Q^T [d, s]
            v_sb = persist.tile([P, NT, D], F32R, name="v_sb")  # V [s, d]

            ident_f = persist.tile([P, P], F32, name="ident_f")
            make_identity(nc, ident_f[:])
            ident_r = persist.tile([P, P], F32R, name="ident_r")
            nc.vector.tensor_copy(ident_r[:], ident_f[:])

            # master causal mask [P, 1024]: cols 0:384 valid, 384:512 causal
            # triangle (row p valid for col j-384 <= p), 512:1024 masked.
            mask = persist.tile([P, 1024], F32, name="mask")
            nc.vector.memset(mask[:, 0:512], 0.0)
            nc.gpsimd.affine_select(
                out=mask[:, 384:512], in_=mask[:, 384:512],
                compare_op=mybir.AluOpType.is_ge, fill=MASKVAL,
                base=0, pattern=[[-1, P]], channel_multiplier=1,
            )
            nc.vector.memset(mask[:, 512:1024], MASKVAL)

            copy_idx = 0

            def copy(out_ap, in_ap):
                # alternate PSUM->SBUF copies between DVE and ACT
                nonlocal copy_idx
                copy_idx += 1
                if copy_idx % 2:
                    nc.vector.tensor_copy(out_ap, in_ap)
                else:
                    nc.scalar.copy(out_ap, in_ap)

            # --- stage 1: transpose x, project K^T/Q^T/V ----------------
            # Software-pipelined: tb+1's x-transposes are emitted interleaved
            # between tb's projection psum-groups so the PE never waits at a
            # t-block boundary. Stage-1 PSUM->SBUF copies run on DVE only
            # (ACT's instruction stream carries the x DMA issues).
            with tc.tile_pool(name="s1", bufs=1) as s1, \
                 tc.tile_pool(name="s1ps", bufs=1, space="PSUM") as s1ps:

                xtbs = {}

                def tr_units(tb):
                    """Yield 16 transpose units (4 MMs + 1 copy each)."""
                    xtb = xtbs[tb]
                    for ti in range(TB // P):
                        tt = tb * (TB // P) + ti
                        on_swdge = tt >= 12
                        x_dt = F32R if on_swdge else F32
                        x_id = ident_r if on_swdge else ident_f
                        x_t = s1.tile([P, E], x_dt, name="x_t", tag="x_t",
                                      bufs=3)
                        if on_swdge:
                            nc.gpsimd.dma_start(x_t[:],
                                                x_ap[tt * P:(tt + 1) * P, :])
                        else:
                            # quarter-split so transposes of each 512-col
                            # group start as soon as that slice lands
                            for q4 in range(4):
                                nc.scalar.dma_start(
                                    x_t[:, q4 * 512:(q4 + 1) * 512],
                                    x_ap[tt * P:(tt + 1) * P,
                                         q4 * 512:(q4 + 1) * 512])
                        for ecg in range(EC // 4):
                            tr_ps = s1ps.tile([P, 4, P], x_dt, name="tr_ps",
                                              tag="tr", bufs=4)
                            for j in range(4):
                                ec = ecg * 4 + j
                                nc.tensor.transpose(
                                    tr_ps[:, j],
                                    x_t[:, ec * P:(ec + 1) * P],
                                    x_id[:])
                            nc.vector.tensor_copy(
                                xtb[:, ecg * 4:(ecg + 1) * 4,
                                    ti * P:(ti + 1) * P],
                                tr_ps[:])
                            yield

                def proj_units(tb):
                    """Yield 8 projection units (16 MMs + 1 copy each)."""
                    xtb = xtbs[tb]
                    for wt, dst in ((wk_r, kt), (wq_r, qt)):
                        for dc in range(2):
                            pp = s1ps.tile([P, TB], F32, name="pp",
                                           tag="proj", bufs=4)
                            for ec in range(EC):
                                nc.tensor.matmul(
                                    pp[:],
                                    wt[:, ec, dc * P:(dc + 1) * P],
                                    xtb[:, ec, :],
                                    start=(ec == 0), stop=(ec == EC - 1))
                            nc.vector.tensor_copy(
                                dst[:, dc, tb * TB:(tb + 1) * TB], pp[:])
                            yield
                    for st in range(TB // P):
                        sg = tb * (TB // P) + st
                        pv = s1ps.tile([P, D], F32, name="pv",
                                       tag="proj", bufs=4)
                        for ec in range(EC):
                            nc.tensor.matmul(
                                pv[:],
                                xtb[:, ec, st * P:(st + 1) * P],
                                wv_r[:, ec, :],
                                start=(ec == 0), stop=(ec == EC - 1))
                        nc.vector.tensor_copy(v_sb[:, sg, :], pv[:])
                        yield

                xtbs[0] = s1.tile([P, EC, TB], F32R, name="xtb", tag="xtb",
                                  bufs=2)
                for _ in tr_units(0):
                    pass
                for tb in range(NTB):
                    if tb + 1 < NTB:
                        xtbs[tb + 1] = s1.tile([P, EC, TB], F32R, name="xtb",
                                               tag="xtb", bufs=2)
                        nxt = tr_units(tb + 1)
                    else:
                        nxt = iter(())
                    for _ in proj_units(tb):
                        next(nxt, None)
                        next(nxt, None)
                    for _ in nxt:
                        pass

            # --- stage 2: causal attention per 128-row query tile -------
            with tc.tile_pool(name="s2", bufs=1) as s2, \
                 tc.tile_pool(name="s2ps", bufs=1, space="PSUM") as s2ps:
                for tt in range(NT):
                    nblk = tt // 4 + 1
                    rem = (tt + 1) * P - (nblk - 1) * SB   # 128..512
                    out_ps = s2ps.tile([P, D], F32, name="out_ps", tag="out",
                                       bufs=3)
                    sums = s2.tile([P, 4], F32, name="sums", tag="sums",
                                   bufs=2)
                    for b in range(nblk):
                        s0 = b * SB
                        last = b == nblk - 1
                        nj = (rem // P) if last else (SB // P)
                        sc_ps = s2ps.tile([P, SB], F32, name="sc_ps",
                                          tag="sc", bufs=3)
                        for dc in range(2):
                            nc.tensor.matmul(
                                sc_ps[:],
                                kt[:, dc, tt * P:(tt + 1) * P],
                                qt[:, dc, s0:s0 + SB],
                                start=(dc == 0), stop=(dc == 1))
                        if last:
                            off = SB - rem
                            nc.vector.tensor_add(sc_ps[:], sc_ps[:],
                                                 mask[:, off:off + SB])
                        p_sb = s2.tile([P, SB], F32R, name="p_sb", tag="p",
                                       bufs=4)
                        nc.scalar.activation(
                            p_sb[:], sc_ps[:],
                            mybir.ActivationFunctionType.Exp,
                            scale=SCALE, accum_out=sums[:, b:b + 1])
                        pt_ps = s2ps.tile([P, 4, P], F32R, name="pt_ps",
                                          tag="pt", bufs=2)
                        for j in range(nj):
                            nc.tensor.transpose(
                                pt_ps[:, j], p_sb[:, j * P:(j + 1) * P],
                                ident_r[:])
                        pt_sb = s2.tile([P, 4, P], F32R, name="pt_sb",
                                        tag="pts", bufs=4)
                        copy(pt_sb[:, 0:nj], pt_ps[:, 0:nj])
                        for j in range(nj):
                            nc.tensor.matmul(
                                out_ps[:], pt_sb[:, j],
                                v_sb[:, b * 4 + j, :],
                                start=(b == 0 and j == 0),
                                stop=(last and j == nj - 1))
                    tot = s2.tile([P, 1], F32, name="tot", tag="tot", bufs=2)
                    nc.vector.reduce_sum(tot[:], sums[:, 0:nblk],
                                         axis=mybir.AxisListType.X)
                    rec = s2.tile([P, 1], F32, name="rec", tag="rec", bufs=2)
                    nc.vector.reciprocal(rec[:], tot[:])
                    o_sb = s2.tile([P, D], F32, name="o_sb", tag="osb", bufs=3)
                    nc.vector.tensor_scalar_mul(o_sb[:], out_ps[:], rec[:])
                    nc.sync.dma_start(y_ap[tt * P:(tt + 1) * P, :], o_sb[:])

    nc.compile()
    return nc


_NC_CACHE = None


def _get_nc():
    global _NC_CACHE
    if _NC_CACHE is None:
        _NC_CACHE = _build()
    return _NC_CACHE


def run(inputs: dict, trace: bool = False):
    """Run on 8 NeuronCores. Returns (out [8,T,D] f32, exec_time_ns|None)."""
    x = np.ascontiguousarray(np.asarray(inputs["x"], dtype=np.float32))
    wk = np.ascontiguousarray(np.asarray(inputs["Wk"], dtype=np.float32))
    wq = np.ascontiguousarray(np.asarray(inputs["Wq"], dtype=np.float32))
    wv = np.ascontiguousarray(np.asarray(inputs["Wv"], dtype=np.float32))
    assert x.shape == (N_CORES, T, E), x.shape

    nc = _get_nc()
    in_maps = [{"x": x[i], "Wk": wk, "Wq": wq, "Wv": wv}
               for i in range(N_CORES)]
    res = run_bass_kernel_spmd(nc, in_maps, core_ids=list(range(N_CORES)),
                               trace=trace)
    out = np.stack([res.results[i]["out"] for i in range(N_CORES)], axis=0)
    return out, res.exec_time_ns


def kernel(**inputs) -> np.ndarray:
    out, _ = run(inputs, trace=False)
    return out
